# revision 1
# baseline (speedup 1.0000x reference)
"""Bass/Tile kernel v9 for nn_BasicGRUClassifier on 8 Trainium2 NeuronCores.

Data-parallel over batch (32 samples/core), [H=128 part, B=32 free] layout.
All matmuls bf16 (fp32 PSUM accumulation); gate preacts live in PSUM
(chunk=8, double-buffered, 8 banks); biases folded into matmuls.

v3 over v2:
  * Phase matmuls (x-projections for the next chunks) are emitted
    interleaved, ~3 per timestep, instead of as a 15-matmul burst at each
    chunk boundary that stalled the in-order PE queue for ~8us.
  * L1 lags L0 by 12 steps (not 8) so the L1 x-projection of a chunk has
    4 steps of slack to drain through the interleaved queue.
  * Sign-folding: u-gate and o-gate weight columns are negated host-side,
    so sigmoid yields u_hat=1-u and tanh yields o'=-o. Then
    m1 = u_hat*h        (GPSIMD tensor_tensor, off critical path)
    m2 = (u_hat-1)*o'   (one DVE scalar_tensor_tensor, critical path)
    h_new = m1 + m2     (DVE, same queue as m2 -> no extra hop)
    This removes v2's 755ns GPSIMD TENSOR_SCALAR from the h-update path.
  * First X DMA piece is only 16 steps so phase1(0) starts early.

v8: m1 = u_hat*h moved from GPSIMD to DVE (the h-add then never waits a
cross-engine semaphore; all four elementwise ops sit in one DVE queue
positioned in natural bubbles). The o-gate bias rides the tanh's
per-partition bias AP (free - ACT converts float biases to APs anyway),
removing the L1 o-bias matmul from each chunk. FC weights are DMA'd
after the X tiles so startup isn't gated on them.

v5: X is transposed host-side to [IN_CH, seq, BL] so the on-chip X tiles
are (c, t, b) t-major: phase1's moving operands are fully contiguous
(620ns strided phase matmuls -> ~330ns) and the DMA is a plain linear
copy. Phase thunks drain 2 per step.
"""

import numpy as np

HID = 128
IN_CH = 271
SEQ = 281
NCLS = 1854
BATCH = 256
NCORES = 8
BL = BATCH // NCORES  # 32 per-core batch
LCH = 8               # timesteps per chunk (PSUM-resident preacts)
LAG = 12              # L1 runs this many steps behind L0
G3 = 3 * HID          # 384

_CACHE = {}


def _build(seq_t):
    import concourse.bass as bass
    import concourse.bacc as bacc
    import concourse.tile as tile
    import concourse.mybir as mybir

    fp32 = mybir.dt.float32
    bf16 = mybir.dt.bfloat16
    AF = mybir.ActivationFunctionType
    ALU = mybir.AluOpType

    nch = (seq_t + LCH - 1) // LCH
    chlen = [min(LCH, seq_t - c * LCH) for c in range(nch)]
    # X DMA time pieces (chunk-aligned); small first piece for fast start
    pb = [0, 16, 144, seq_t]
    npieces = len(pb) - 1

    nc = bacc.Bacc()
    X = nc.dram_tensor("X", [IN_CH, seq_t, BL], bf16, kind="ExternalInput")
    Wx0a = nc.dram_tensor("Wx0a", [HID, G3], bf16, kind="ExternalInput")
    Wx0b = nc.dram_tensor("Wx0b", [HID, G3], bf16, kind="ExternalInput")
    Wx0c = nc.dram_tensor("Wx0c", [16, G3], bf16, kind="ExternalInput")  # bias row + 15 ch
    Uh0 = nc.dram_tensor("Uh0", [HID, G3], bf16, kind="ExternalInput")
    Wx1 = nc.dram_tensor("Wx1", [HID, G3], bf16, kind="ExternalInput")
    Uh1 = nc.dram_tensor("Uh1", [HID, G3], bf16, kind="ExternalInput")
    B1 = nc.dram_tensor("B1", [1, G3], bf16, kind="ExternalInput")
    BO0 = nc.dram_tensor("BO0", [HID, 1], fp32, kind="ExternalInput")
    BO1 = nc.dram_tensor("BO1", [HID, 1], fp32, kind="ExternalInput")
    WFC = nc.dram_tensor("WFC", [HID, NCLS], bf16, kind="ExternalInput")
    BFC = nc.dram_tensor("BFC", [1, NCLS], bf16, kind="ExternalInput")
    OUT = nc.dram_tensor("OUT", [BL, NCLS], fp32, kind="ExternalOutput")

    with tile.TileContext(nc) as tc:
        from contextlib import ExitStack
        with ExitStack() as ctx:
            const = ctx.enter_context(tc.tile_pool(name="const", bufs=1))
            rus = ctx.enter_context(tc.tile_pool(name="rus", bufs=3))
            rhs_p = ctx.enter_context(tc.tile_pool(name="rhp", bufs=3))
            os_p = ctx.enter_context(tc.tile_pool(name="osp", bufs=3))
            m1p = ctx.enter_context(tc.tile_pool(name="m1p", bufs=3))
            m2p = ctx.enter_context(tc.tile_pool(name="m2p", bufs=3))
            h0sq = ctx.enter_context(tc.tile_pool(name="h0sq", bufs=2))
            h1p = ctx.enter_context(tc.tile_pool(name="h1p", bufs=3))
            outp = ctx.enter_context(tc.tile_pool(name="outp", bufs=1))
            # PSUM pools: 4 pools x 2 bufs = 8 banks
            pru0 = ctx.enter_context(tc.tile_pool(name="pru0", bufs=2, space="PSUM"))
            po0 = ctx.enter_context(tc.tile_pool(name="po0", bufs=2, space="PSUM"))
            pru1 = ctx.enter_context(tc.tile_pool(name="pru1", bufs=2, space="PSUM"))
            po1 = ctx.enter_context(tc.tile_pool(name="po1", bufs=2, space="PSUM"))

            # ---- constants into SBUF ----
            def load_const(name, dram, shape):
                t_ = const.tile(shape, bf16, tag=name)
                nc.sync.dma_start(out=t_, in_=dram[:, :])
                return t_

            wx0_sb = [
                load_const("wx0a", Wx0a, [HID, G3]),
                load_const("wx0b", Wx0b, [HID, G3]),
                load_const("wx0c", Wx0c, [16, G3]),
            ]
            uh0_sb = load_const("uh0", Uh0, [HID, G3])
            wx1_sb = load_const("wx1", Wx1, [HID, G3])
            uh1_sb = load_const("uh1", Uh1, [HID, G3])
            b1_sb = load_const("b1", B1, [1, G3])
            bo_sb = []
            for nm, dram in (("bo0", BO0), ("bo1", BO1)):
                t_ = const.tile([HID, 1], fp32, tag=nm)
                nc.sync.dma_start(out=t_, in_=dram[:, :])
                bo_sb.append(t_)
            ones_sb = const.tile([1, 512], bf16, tag="ones")
            nc.vector.memset(ones_sb, 1.0)

            # X resident tiles, one per (k-tile, time-piece).
            ksz = [128, 128, 15]
            kof = [0, 128, 256]
            xt_sb = {}
            for p in range(npieces):
                for k in range(3):
                    part = 16 if k == 2 else ksz[k]
                    plen = pb[p + 1] - pb[p]
                    t_ = const.tile([part, BL * plen], bf16, tag=f"xt_{k}_{p}")
                    srcv = X[kof[k]:kof[k] + ksz[k], pb[p]:pb[p + 1], :].rearrange(
                        "c t b -> c (t b)")
                    # k==2: row 0 is the ones row (bias); X channels at rows 1..15
                    r0 = 1 if k == 2 else 0
                    nc.sync.dma_start(out=t_[r0:r0 + ksz[k], :], in_=srcv)
                    if k == 2:
                        nc.vector.memset(t_[0:1, :], 1.0)
                    xt_sb[(k, p)] = t_

            wfc_sb = load_const("wfc", WFC, [HID, NCLS])
            bfc_sb = load_const("bfc", BFC, [1, NCLS])

            def piece_of(ch):
                t0 = ch * LCH
                for p in range(npieces):
                    if t0 < pb[p + 1]:
                        return p
                return npieces - 1

            # ---------- phase emitters (return (tiles, thunks)) ----------
            def phase1(ch):
                """L0 x-preacts for chunk ch -> PSUM (9 matmul thunks)."""
                Lc = chlen[ch]
                p = piece_of(ch)
                t0 = ch * LCH - pb[p]
                ru = pru0.tile([HID, 512], fp32, tag="pru0")
                o = po0.tile([HID, 256], fp32, tag="po0")
                ruv = ru.rearrange("p (t z) -> p t z", t=LCH)
                ov = o.rearrange("p (t b) -> p t b", t=LCH)
                dsts = [ruv[:, 0:Lc, 0:32], ruv[:, 0:Lc, 32:64], ov[:, 0:Lc, :]]
                thunks = []
                for g in range(3):
                    for k in range(3):
                        def th(g=g, k=k):
                            xv = xt_sb[(k, p)].rearrange(
                                "c (t b) -> c t b", b=BL)[:, t0:t0 + Lc, :]
                            nc.tensor.matmul(
                                dsts[g],
                                wx0_sb[k][:, g * HID:(g + 1) * HID],
                                xv,
                                start=(g == 0 and k == 0) or (g == 2 and k == 0),
                                stop=(k == 2),
                            )
                        thunks.append(th)
                return (ru, o), thunks

            def phase3(ch, h0seq):
                """L1 x-preacts for chunk ch from h0seq (6 matmul thunks)."""
                Lc = chlen[ch]
                ru = pru1.tile([HID, 512], fp32, tag="pru1")
                o = po1.tile([HID, 256], fp32, tag="po1")
                ruv = ru.rearrange("p (t z) -> p t z", t=LCH)
                ov = o.rearrange("p (t b) -> p t b", t=LCH)
                dsts = [ruv[:, 0:Lc, 0:32], ruv[:, 0:Lc, 32:64], ov[:, 0:Lc, :]]
                hv = h0seq[:, 0:Lc * BL].rearrange("p (t b) -> p t b", t=Lc)
                onev = ones_sb[:, 0:Lc * BL].rearrange("p (t b) -> p t b", t=Lc)
                thunks = []
                for g in range(3):
                    def th1(g=g):
                        nc.tensor.matmul(
                            dsts[g], wx1_sb[:, g * HID:(g + 1) * HID], hv,
                            start=(g == 0) or (g == 2), stop=False)
                    def th2(g=g):
                        nc.tensor.matmul(
                            dsts[g], b1_sb[:, g * HID:(g + 1) * HID], onev,
                            start=False, stop=True)
                    thunks += [th1, th2]
                return (ru, o), thunks

            # ---------- GRU cell ----------
            def cell(lyr, ru_ps, o_ps, tl, uh, st, h_out):
                """One GRU cell with sign-folded u/o gates.
                sigmoid -> [r | u_hat=1-u]; tanh -> o' = -o."""
                rur = ru_ps[:, tl * 64:tl * 64 + 32]
                ruu = ru_ps[:, tl * 64 + 32:tl * 64 + 64]
                h = st["h"]
                Ur = uh[:, 0:HID]
                Uu = uh[:, HID:2 * HID]
                Uo = uh[:, 2 * HID:3 * HID]
                if h is not None:
                    nc.tensor.matmul(rur, Ur, h, start=False, stop=True)
                    nc.tensor.matmul(ruu, Uu, h, start=False, stop=True)
                ru_sb = rus.tile([HID, 64], bf16, tag=f"ru{lyr}")
                nc.scalar.activation(ru_sb, ru_ps[:, tl * 64:(tl + 1) * 64], AF.Sigmoid)
                if h is not None:
                    rh = rhs_p.tile([HID, BL], bf16, tag=f"rh{lyr}")
                    nc.vector.tensor_mul(rh, ru_sb[:, 0:32], h)
                    nc.tensor.matmul(o_ps[:, tl * 32:(tl + 1) * 32], Uo, rh,
                                     start=False, stop=True)
                    # off-critical-path: m1 = u_hat * h (DVE, sits in the
                    # sigmoid->tanh bubble; keeps h-add same-queue)
                    m1 = m1p.tile([HID, BL], bf16, tag=f"m1{lyr}")
                    nc.vector.tensor_mul(m1, ru_sb[:, 32:64], h)
                o_sb = os_p.tile([HID, BL], bf16, tag=f"o{lyr}")
                nc.scalar.activation(o_sb, o_ps[:, tl * 32:(tl + 1) * 32], AF.Tanh)
                # critical path: m2 = (u_hat-1)*o' = u*o ; h_new = m1 + m2
                if h is not None:
                    m2 = m2p.tile([HID, BL], bf16, tag=f"m2{lyr}")
                    nc.vector.scalar_tensor_tensor(
                        m2, ru_sb[:, 32:64], 1.0, o_sb, ALU.subtract, ALU.mult)
                    nc.vector.tensor_add(h_out, m1, m2)
                else:
                    # h_0 = u*o = (u_hat-1)*o' written straight to h_out
                    nc.vector.scalar_tensor_tensor(
                        h_out, ru_sb[:, 32:64], 1.0, o_sb, ALU.subtract, ALU.mult)
                st["h"] = h_out

            # ---------- main pipeline ----------
            st0 = {"h": None}
            st1 = {"h": None}
            ps0 = {}
            ps1_ = {}
            pending = []

            def drain(n):
                for _ in range(min(n, len(pending))):
                    pending.pop(0)()

            ps0[0], th = phase1(0)
            for t_ in th:
                t_()
            if nch > 1:
                ps0[1], th = phase1(1)
                for t_ in th:
                    t_()

            def l1_step(tg):
                pch, tl = tg // LCH, tg % LCH
                ru, o = ps1_[pch]
                h1n = h1p.tile([HID, BL], bf16, tag="h1")
                cell(1, ru, o, tl, uh1_sb, st1, h1n[:, :])

            for ch in range(nch):
                Lc = chlen[ch]
                h0seq = h0sq.tile([HID, LCH * BL], bf16, tag="h0seq")
                ru0, o0 = ps0[ch]
                for tl in range(Lc):
                    tg = ch * LCH + tl
                    cell(0, ru0, o0, tl, uh0_sb, st0,
                         h0seq[:, tl * BL:(tl + 1) * BL])
                    if tg - LAG >= 0:
                        l1_step(tg - LAG)
                    drain(2)
                ps0.pop(ch)
                ps1_[ch], th3 = phase3(ch, h0seq)
                pending += th3
                if ch + 2 < nch:
                    ps0[ch + 2], th1 = phase1(ch + 2)
                    pending += th1

            drain(len(pending))
            for tg in range(max(0, seq_t - LAG), seq_t):
                l1_step(tg)
            h1_fin = st1["h"]

            # ---------- FC ----------
            out_sb = outp.tile([BL, NCLS], fp32, tag="osb")
            nsl = [512, 512, 512, NCLS - 3 * 512]
            for i in range(4):
                n0 = i * 512
                ps = pru0.tile([HID, 512], fp32, tag="pru0")
                pf = ps[:BL, :nsl[i]]
                nc.tensor.matmul(pf, ones_sb[:, 0:BL], bfc_sb[:, n0:n0 + nsl[i]],
                                 start=True, stop=False)
                nc.tensor.matmul(pf, h1_fin, wfc_sb[:, n0:n0 + nsl[i]],
                                 start=False, stop=True)
                nc.vector.tensor_scalar_add(out_sb[:, n0:n0 + nsl[i]], pf, 0.0)
            nc.sync.dma_start(out=OUT[:, :], in_=out_sb)

    nc.finalize()
    return nc


def _to_bf16(a):
    import ml_dtypes
    return np.ascontiguousarray(a.astype(ml_dtypes.bfloat16))


def _neg_uo(W):
    """Negate the u-gate and o-gate column blocks of a [*, 384] matrix."""
    W = W.copy()
    W[:, HID:] = -W[:, HID:]
    return W


def _prep_consts(inputs):
    Wx0 = np.concatenate(
        [inputs["Wr0"][:IN_CH], inputs["Wu0"][:IN_CH], inputs["Wo0"][:IN_CH]], axis=1)
    b0row = np.concatenate([inputs["br0"], inputs["bu0"], inputs["bo0"]])[None, :]
    Uh0 = np.concatenate(
        [inputs["Wr0"][IN_CH:], inputs["Wu0"][IN_CH:], inputs["Wo0"][IN_CH:]], axis=1)
    Wx1 = np.concatenate(
        [inputs["Wr1"][:HID], inputs["Wu1"][:HID], inputs["Wo1"][:HID]], axis=1)
    Uh1 = np.concatenate(
        [inputs["Wr1"][HID:], inputs["Wu1"][HID:], inputs["Wo1"][HID:]], axis=1)
    B1 = np.concatenate([inputs["br1"], inputs["bu1"], inputs["bo1"]])[None, :]
    Wx0 = _neg_uo(Wx0)
    b0row = _neg_uo(b0row)
    Uh0 = _neg_uo(Uh0)
    Wx1 = _neg_uo(Wx1)
    B1 = _neg_uo(B1)
    Uh1 = _neg_uo(Uh1)
    BO0 = np.zeros((HID, 1), np.float32)
    BO1 = np.zeros((HID, 1), np.float32)
    return dict(
        Wx0a=_to_bf16(Wx0[0:128]),
        Wx0b=_to_bf16(Wx0[128:256]),
        Wx0c=_to_bf16(np.concatenate([b0row, Wx0[256:271]], axis=0)),
        Uh0=_to_bf16(Uh0),
        Wx1=_to_bf16(Wx1),
        Uh1=_to_bf16(Uh1),
        B1=_to_bf16(B1),
        BO0=BO0,
        BO1=BO1,
        WFC=_to_bf16(inputs["Wfc"]),
        BFC=_to_bf16(inputs["bfc"][None, :]),
    )


def kernel(_trace=False, **inputs):
    from concourse.bass_utils import run_bass_kernel_spmd

    seq_t = inputs["X"].shape[2]
    if "nc" not in _CACHE or _CACHE.get("seq_t") != seq_t:
        _CACHE["nc"] = _build(seq_t)
        _CACHE["seq_t"] = seq_t
    nc = _CACHE["nc"]

    consts = _prep_consts(inputs)
    Xb = _to_bf16(np.asarray(inputs["X"]))
    in_maps = []
    for c in range(NCORES):
        m = dict(consts)
        # [BL, C, T] -> [C, T, BL] so on-chip tiles are t-major contiguous
        m["X"] = np.ascontiguousarray(Xb[c * BL:(c + 1) * BL].transpose(1, 2, 0))
        in_maps.append(m)

    res = run_bass_kernel_spmd(nc, in_maps, core_ids=list(range(NCORES)), trace=_trace)
    out = np.concatenate([r["OUT"] for r in res.results], axis=0)
    if _trace:
        _CACHE["last_exec_time_ns"] = res.exec_time_ns
        _CACHE["last_profile"] = res.profile_json
        _CACHE["last_trace"] = res.instructions_and_trace
    return out

